# revision 24
# baseline (speedup 1.0000x reference)
"""Trainium2 Bass kernel for nn_DGCNConv (DGCNN-style GNN with sortpooling).

Strategy (data-parallel over output rows / graphs, 8 cores):
  - Host builds the dense symmetrized adjacency A_T (entries are small
    integers -> EXACT in bf16) and the degree scalings d1 = d_in^-1/2,
    d2 = d_out^-1/2.
  - Each core owns 1024 rows (4 graphs). The big matmuls A_T @ S are row
    sharded: core c computes rows [1024c, 1024c+1024) and needs the full S,
    which is produced by chunked AllGathers that overlap the previous
    round's matmul stream.
  - Algebra: using symmetry of A_T and commutation of row-scaling with
    right-multiplication:
        cur' = tanh(d .* (A_T @ S) + b),   S = d .* (cur @ W^T)
    so the tiny feature matmul (128x128) happens BEFORE the big adjacency
    matmul, and all scalings are per-partition ops.
  - Precision: single-pass bf16 streams everywhere EXCEPT (a) the round-3
    cur stream (which feeds the top-k rank channel through only one more
    layer) gets a bf16 hi/lo double pass, and (b) the final-layer rank
    channel itself (feat[:, -1]) gets an extra bf16 lo-residual column.
    Errors injected in rounds 0-2 attenuate ~10x per layer through the
    normalized graph convolutions; host simulation of this exact scheme on
    the real inputs gives rel err ~6e-3 (gate 2e-2) with a handful of
    top-k near-tie swaps.
  - Stage 0 (S from the input x) is computed on the host and fed as a
    broadcast input, removing one device round + collective.
  - Per-graph top-k (k=60) sortpooling is done on-device via rank
    computation (comparison counting with stable tie-break) and a
    permutation-matrix matmul gather.
"""

import numpy as np
import ml_dtypes

# ---- problem constants (hardcoded per contract) ----
N = 8192          # nodes
F = 128           # feature dim
L = 4             # layers
G = 32            # graphs
NPG = 256         # nodes per graph
K = 60            # sortpool k
NC = 8            # cores
P = 128           # partitions
ROWS = N // NC    # 1024 rows per core
IT = ROWS // P    # 8 i-tiles per core
JT = N // P       # 64 j-tiles
D = L * F         # 512 output feature dim
GPC = G // NC     # 4 graphs per core

BF16 = ml_dtypes.bfloat16

_CACHE = {}

# per-round stage widths (bf16 cols).  The cur2 branch carries 0.9 of the
# combine weight (vs 0.1 for cur1), so S2 streams bf16 hi/lo (exact) while
# S1 streams single-pass bf16:
#   r0   [S1 | S2h | S2l]                      384
#   r1-2 [S3 S4 | S1 | S2h | S2l]              640
#   r3   [S3 S4 | S1h S2h | S1l S2l]           768   (S1 also hi/lo: feeds
#                                                      the rank channel with
#                                                      only 1 layer between)
#   r4   [S3h S4h | lo127_3 lo127_4]           258
SR = [384, 640, 640, 768, 258]


def _build_nc(use_cc=True, body_reps=1, gather_chunks=4):
    """Build + compile the Bass program (shared SPMD binary for all 8 cores)."""
    import concourse.bass as bass
    import concourse.bacc as bacc
    import concourse.mybir as mybir
    import concourse.tile as tile

    dt = mybir.dt
    Alu = mybir.AluOpType
    Act = mybir.ActivationFunctionType
    X = mybir.AxisListType.X
    f32 = dt.float32
    bf16 = dt.bfloat16

    CH = gather_chunks
    ITC = IT // CH                  # i-tiles per gather chunk

    nc = bacc.Bacc(
        "TRN2",
        target_bir_lowering=False,
        debug=False,
        enable_asserts=False,
        num_devices=NC,
    )

    # ---------------- I/O ----------------
    at_in = nc.dram_tensor("at", [IT, P, JT * P], bf16, kind="ExternalInput")
    s0_in = nc.dram_tensor("s0", [N, 3 * P], bf16, kind="ExternalInput")
    w1t_in = nc.dram_tensor("w1t", [L, P, P], f32, kind="ExternalInput")
    w2t_in = nc.dram_tensor("w2t", [L, P, P], f32, kind="ExternalInput")
    w3t_in = nc.dram_tensor("w3t", [L, P, P], f32, kind="ExternalInput")
    b1_in = nc.dram_tensor("b1bc", [L, P, P], f32, kind="ExternalInput")
    b2_in = nc.dram_tensor("b2bc", [L, P, P], f32, kind="ExternalInput")
    b3_in = nc.dram_tensor("b3bc", [L, P, P], f32, kind="ExternalInput")
    d1_in = nc.dram_tensor("d1m", [P, IT], f32, kind="ExternalInput")
    d2_in = nc.dram_tensor("d2m", [P, IT], f32, kind="ExternalInput")
    ident_in = nc.dram_tensor("ident", [P, P], f32, kind="ExternalInput")
    iota_in = nc.dram_tensor("iotar", [P, P], f32, kind="ExternalInput")
    ltc_in = nc.dram_tensor("ltc", [2, P, NPG], f32, kind="ExternalInput")
    ones_in = nc.dram_tensor("ones1", [1, P], f32, kind="ExternalInput")
    out_t = nc.dram_tensor("out", [GPC * K, D], f32, kind="ExternalOutput")

    rg = [list(range(NC))]

    with tile.TileContext(nc) as tc:
        with (
            tc.tile_pool(name="const", bufs=1) as cp,
            tc.tile_pool(name="dram", bufs=1, space="DRAM") as dp,
            tc.tile_pool(name="sgath", bufs=64) as sgp,
            tc.tile_pool(name="atp", bufs=2) as atp,
            tc.tile_pool(name="stgp", bufs=3) as stgp,
            tc.tile_pool(name="pbig", bufs=2, space="PSUM") as pbig,
            tc.tile_pool(name="psm", bufs=5, space="PSUM") as psm,
            tc.tile_pool(name="tmp", bufs=3) as tmpp,
            tc.tile_pool(name="curp", bufs=3) as curp,
        ):
            # ---- persistent constants / state ----
            def const_tile(shape, nm):
                return cp.tile(shape, f32, name=nm, tag=nm)

            ident = const_tile([P, P], "ident_sb")
            iotar = const_tile([P, P], "iotar_sb")
            ltc = const_tile([P, 2 * NPG], "ltc_sb")
            ones1 = const_tile([1, P], "ones1_sb")
            w1t = const_tile([P, L * P], "w1t_sb")
            w2t = const_tile([P, L * P], "w2t_sb")
            w3t = const_tile([P, L * P], "w3t_sb")
            b1bc = const_tile([P, L * P], "b1bc_sb")
            b2bc = const_tile([P, L * P], "b2bc_sb")
            b3bc = const_tile([P, L * P], "b3bc_sb")
            d1m = const_tile([P, IT], "d1m_sb")
            d2m = const_tile([P, IT], "d2m_sb")
            feat = const_tile([P, IT * D], "feat_sb")

            nc.sync.dma_start(ident[:], ident_in[:])
            nc.sync.dma_start(iotar[:], iota_in[:])
            nc.sync.dma_start(ltc[:, 0:NPG], ltc_in[0])
            nc.sync.dma_start(ltc[:, NPG:2 * NPG], ltc_in[1])
            nc.sync.dma_start(ones1[:], ones_in[:])
            for lv in range(L):
                sl = slice(lv * P, (lv + 1) * P)
                nc.sync.dma_start(w1t[:, sl], w1t_in[lv])
                nc.sync.dma_start(w2t[:, sl], w2t_in[lv])
                nc.sync.dma_start(w3t[:, sl], w3t_in[lv])
                nc.sync.dma_start(b1bc[:, sl], b1_in[lv])
                nc.sync.dma_start(b2bc[:, sl], b2_in[lv])
                nc.sync.dma_start(b3bc[:, sl], b3_in[lv])
            nc.sync.dma_start(d1m[:], d1_in[:])
            nc.sync.dma_start(d2m[:], d2_in[:])

            _repc = [0]

            def emit_body():
                rep = _repc[0]
                _repc[0] += 1
                # DRAM bounce buffers for the chunked AllGathers (fresh per
                # rep: Shared tensors allow only a single writer instruction)
                stage_c = {}
                gath_c = {}
                for r in range(1, 5):
                    for k in range(CH):
                        stage_c[(r, k)] = dp.tile(
                            [ITC * P, SR[r]], bf16, name=f"stg{r}_{k}_{rep}",
                            tag=f"stg{r}_{k}_{rep}")
                        gath_c[(r, k)] = dp.tile(
                            [NC * ITC * P, SR[r]], bf16,
                            name=f"gth{r}_{k}_{rep}",
                            tag=f"gth{r}_{k}_{rep}", addr_space="Shared")

                def combine(dst, a, b):
                    """dst = 0.5*(0.1*a + 0.9*b), matching reference rounding."""
                    t1 = tmpp.tile([P, P], f32, tag="comb1")
                    nc.vector.tensor_scalar_mul(t1[:], a[:], 0.1)
                    t2 = tmpp.tile([P, P], f32, tag="comb2")
                    nc.vector.scalar_tensor_tensor(t2[:], b[:], 0.9, t1[:],
                                                   Alu.mult, Alu.add)
                    nc.vector.tensor_scalar_mul(dst, t2[:], 0.5)

                # ---------------- sortpooling ----------------
                def emit_pool(g):
                    # channel values of the two node tiles of this graph
                    repl = []
                    for tf in range(2):
                        it = 2 * g + tf
                        col = feat[:, it * D + D - 1:it * D + D]   # [P, 1]
                        vtp = psm.tile([1, P], f32, tag="smallps")
                        nc.tensor.transpose(vtp[:], col, ident[:])
                        vrow = tmpp.tile([1, P], f32, tag="vrow")
                        nc.vector.tensor_copy(vrow[:], vtp[:])
                        rp = psm.tile([P, P], f32, tag="smallps")
                        nc.tensor.matmul(rp[:], ones1[:], vrow[:],
                                         start=True, stop=True)
                        repl.append(rp)
                    poolps = pbig.tile([P, D], f32, tag="pbig")
                    for tp in range(2):
                        it = 2 * g + tp
                        vcol = feat[:, it * D + D - 1:it * D + D]
                        Ct = tmpp.tile([P, NPG], f32, tag="Ct")
                        Et = tmpp.tile([P, NPG], f32, tag="Et")
                        for tf in range(2):
                            sl = slice(tf * P, (tf + 1) * P)
                            nc.vector.tensor_scalar(Ct[:, sl], repl[tf][:], vcol,
                                                    None, Alu.is_gt)
                            nc.vector.tensor_scalar(Et[:, sl], repl[tf][:], vcol,
                                                    None, Alu.is_equal)
                        nc.vector.tensor_tensor(
                            Et[:], Et[:], ltc[:, tp * NPG:(tp + 1) * NPG],
                            Alu.mult)
                        nc.vector.tensor_tensor(Ct[:], Ct[:], Et[:], Alu.add)
                        ranks = tmpp.tile([P, 1], f32, tag="ranks")
                        nc.vector.reduce_sum(ranks[:], Ct[:], axis=X)
                        perm = curp.tile([P, P], f32, tag="perm")
                        nc.vector.tensor_scalar(perm[:], iotar[:], ranks[:],
                                                None, Alu.is_equal)
                        nc.tensor.matmul(poolps[:], perm[:],
                                         feat[:, it * D:(it + 1) * D],
                                         start=(tp == 0), stop=(tp == 1))
                    osb = tmpp.tile([P, D], f32, tag="osb")
                    nc.vector.tensor_copy(osb[0:K, :], poolps[0:K, :])
                    nc.sync.dma_start(out_t[g * K:(g + 1) * K, :], osb[0:K, :])

                # ---------------- rounds ----------------
                # jt consumption order: chunk-major so early gather chunks
                # unblock the matmul stream before late chunks arrive.
                ORDER = [8 * c + k * ITC + i
                         for k in range(CH) for c in range(NC)
                         for i in range(ITC)]

                def post_it(r, it, ps):
                    has_cats = r >= 1
                    has_curs = r <= 3
                    lv_cat = r - 1
                    lv_cur = r
                    if True:
                        d1c = d1m[:, it:it + 1]
                        d2c = d2m[:, it:it + 1]

                        if r == 0:
                            # fold A@S2l (cols 256:384) into cur2 (128:256)
                            # via an SBUF bounce (DVE reads only 1 PSUM input)
                            tl = tmpp.tile([P, P], f32, tag="fold")
                            nc.vector.tensor_copy(tl[:], ps[:, 2 * P:3 * P])
                            nc.vector.tensor_tensor(
                                ps[:, P:2 * P], ps[:, P:2 * P], tl[:], Alu.add)
                        if r == 4:
                            # fold the lo-residual of the rank channel into
                            # the hi accumulation (cols 127 / 255)
                            tl = tmpp.tile([P, 2], f32, tag="fold2")
                            nc.vector.tensor_copy(tl[:], ps[:, 256:258])
                            nc.vector.tensor_tensor(
                                ps[:, 127:128], ps[:, 127:128], tl[:, 0:1],
                                Alu.add)
                            nc.vector.tensor_tensor(
                                ps[:, 255:256], ps[:, 255:256], tl[:, 1:2],
                                Alu.add)

                        def act_group(goff, dcol, bias_sb, lv):
                            pre = tmpp.tile([P, P], f32, tag="pre")
                            nc.vector.scalar_tensor_tensor(
                                pre[:], ps[:, goff:goff + P], dcol,
                                bias_sb[:, lv * P:(lv + 1) * P],
                                Alu.mult, Alu.add)
                            o = curp.tile([P, P], f32, tag="actout")
                            nc.scalar.activation(o[:], pre[:], Act.Tanh)
                            return o

                        goff = 0
                        if has_cats:
                            c3 = act_group(0, d1c, b3bc, lv_cat)
                            c4 = act_group(P, d2c, b3bc, lv_cat)
                            combine(feat[:, it * D + lv_cat * P:
                                          it * D + (lv_cat + 1) * P], c3, c4)
                            goff = 2 * P

                        if has_curs:
                            cur1 = act_group(goff, d1c, b1bc, lv_cur)
                            cur2 = act_group(goff + P, d2c, b2bc, lv_cur)

                            # transposes for the next small matmuls
                            t1p = psm.tile([P, P], f32, tag="smallps")
                            nc.tensor.transpose(t1p[:], cur1[:], ident[:])
                            cur1T = curp.tile([P, P], f32, tag="curT")
                            nc.vector.tensor_copy(cur1T[:], t1p[:])
                            t2p = psm.tile([P, P], f32, tag="smallps")
                            nc.tensor.transpose(t2p[:], cur2[:], ident[:])
                            cur2T = curp.tile([P, P], f32, tag="curT")
                            nc.vector.tensor_copy(cur2T[:], t2p[:])
                            outT = curp.tile([P, P], f32, tag="outT")
                            combine(outT[:], cur1T, cur2T)

                            # small matmuls -> stage r+1
                            rn = r + 1
                            k = it // ITC
                            stg = stgp.tile([P, SR[rn]], bf16, tag="stg")
                            r3z = psm.tile([P, P], f32, tag="smallps")
                            nc.tensor.matmul(
                                r3z[:], outT[:],
                                w3t[:, lv_cur * P:(lv_cur + 1) * P],
                                start=True, stop=True)
                            if rn == 4:
                                # hi parts + lo residual of col 127 only
                                t3 = tmpp.tile([P, P], f32, tag="hilo_f32")
                                nc.vector.tensor_scalar_mul(t3[:], r3z[:], d1c)
                                nc.vector.tensor_copy(stg[:, 0:P], t3[:])
                                nc.vector.scalar_tensor_tensor(
                                    stg[:, 256:257], stg[:, 127:128], -1.0,
                                    t3[:, 127:128], Alu.mult, Alu.add)
                                t4 = tmpp.tile([P, P], f32, tag="hilo_f32")
                                nc.vector.tensor_scalar_mul(t4[:], r3z[:], d2c)
                                nc.vector.tensor_copy(stg[:, P:2 * P], t4[:])
                                nc.vector.scalar_tensor_tensor(
                                    stg[:, 257:258], stg[:, P + 127:P + 128],
                                    -1.0, t4[:, 127:128], Alu.mult, Alu.add)
                            else:
                                # cat part: cols 0:256
                                nc.vector.tensor_scalar_mul(
                                    stg[:, 0:P], r3z[:], d1c)
                                nc.vector.tensor_scalar_mul(
                                    stg[:, P:2 * P], r3z[:], d2c)
                                r1n = psm.tile([P, P], f32, tag="smallps")
                                nc.tensor.matmul(
                                    r1n[:], cur1T[:],
                                    w1t[:, (lv_cur + 1) * P:(lv_cur + 2) * P],
                                    start=True, stop=True)
                                r2n = psm.tile([P, P], f32, tag="smallps")
                                nc.tensor.matmul(
                                    r2n[:], cur2T[:],
                                    w2t[:, (lv_cur + 1) * P:(lv_cur + 2) * P],
                                    start=True, stop=True)
                                if rn == 3:
                                    # S1 and S2 both hi/lo:
                                    # [S1h S2h | S1l S2l] at 256:512 / 512:768
                                    t1 = tmpp.tile([P, P], f32, tag="hilo_f32")
                                    nc.vector.tensor_scalar_mul(
                                        t1[:], r1n[:], d1c)
                                    nc.vector.tensor_copy(
                                        stg[:, 2 * P:3 * P], t1[:])
                                    nc.vector.scalar_tensor_tensor(
                                        stg[:, 4 * P:5 * P],
                                        stg[:, 2 * P:3 * P], -1.0, t1[:],
                                        Alu.mult, Alu.add)
                                    t2 = tmpp.tile([P, P], f32, tag="hilo_f32")
                                    nc.vector.tensor_scalar_mul(
                                        t2[:], r2n[:], d2c)
                                    nc.vector.tensor_copy(
                                        stg[:, 3 * P:4 * P], t2[:])
                                    nc.vector.scalar_tensor_tensor(
                                        stg[:, 5 * P:6 * P],
                                        stg[:, 3 * P:4 * P], -1.0, t2[:],
                                        Alu.mult, Alu.add)
                                else:
                                    # S1 single-pass: cols 256:384
                                    nc.vector.tensor_scalar_mul(
                                        stg[:, 2 * P:3 * P], r1n[:], d1c)
                                    # S2 hi/lo: cols 384:512 / 512:640
                                    t2 = tmpp.tile([P, P], f32, tag="hilo_f32")
                                    nc.vector.tensor_scalar_mul(
                                        t2[:], r2n[:], d2c)
                                    nc.vector.tensor_copy(
                                        stg[:, 3 * P:4 * P], t2[:])
                                    nc.vector.scalar_tensor_tensor(
                                        stg[:, 4 * P:5 * P],
                                        stg[:, 3 * P:4 * P], -1.0, t2[:],
                                        Alu.mult, Alu.add)
                            nc.sync.dma_start(
                                stage_c[(rn, k)][(it - k * ITC) * P:
                                                 (it - k * ITC + 1) * P, :],
                                stg[:])
                            # fire the chunk's AllGather once its last stage
                            # row is written (overlaps remaining matmuls)
                            if use_cc and (it + 1) % ITC == 0:
                                nc.gpsimd.collective_compute(
                                    "AllGather", Alu.bypass, replica_groups=rg,
                                    ins=[stage_c[(rn, k)].opt()],
                                    outs=[gath_c[(rn, k)].opt()])

                for r in range(5):
                    # load this round's S tiles to SBUF
                    s_tiles = [None] * JT
                    if r == 0:
                        for jt in range(JT):
                            st = sgp.tile([P, SR[0]], bf16, tag="s")
                            nc.sync.dma_start(
                                st[:], s0_in[jt * P:(jt + 1) * P, :])
                            s_tiles[jt] = st
                    else:
                        for k in range(CH):
                            for c in range(NC):
                                for i in range(ITC):
                                    jt = 8 * c + k * ITC + i
                                    st = sgp.tile([P, SR[r]], bf16, tag="s")
                                    row = (c * ITC + i) * P
                                    if use_cc:
                                        src = gath_c[(r, k)][row:row + P, :]
                                    else:
                                        # timing-only variant: wrong data,
                                        # same traffic shape
                                        src = stage_c[(r, k)][
                                            (i % ITC) * P:(i % ITC + 1) * P, :]
                                    nc.sync.dma_start(st[:], src)
                                    s_tiles[jt] = st

                    has_cats = r >= 1
                    has_curs = r <= 3
                    lv_cat = r - 1   # W3/b3 layer for the cats of this round
                    lv_cur = r       # W1/W2/b1/b2 layer for the cur update
                    mmw = min(SR[r], 512)

                    pend = None   # previous it's act/stage closure: the PE
                    # queue is FIFO, so emit it-1's transposes/small matmuls
                    # AFTER it's big stream — the activation chain they wait
                    # on then completes behind the big stream (no PE stall).
                    for it in range(IT):
                        strip = atp.tile([P, JT * P], bf16, tag="at")
                        nc.sync.dma_start(strip[:], at_in[it])

                        ps = pbig.tile([P, 4 * P], f32, tag="pbig")
                        nmm = len(ORDER)
                        for mi, jt in enumerate(ORDER):
                            lhsT = strip[:, jt * P:(jt + 1) * P]
                            first = mi == 0
                            last = mi == nmm - 1
                            nc.tensor.matmul(
                                ps[:, 0:mmw], lhsT, s_tiles[jt][:, 0:mmw],
                                start=first, stop=(last if SR[r] <= 512
                                                   else False))
                            if SR[r] > 512:   # lo pass -> tail psum region
                                low = SR[r] - 512
                                nc.tensor.matmul(
                                    ps[:, 4 * P - low:4 * P], lhsT,
                                    s_tiles[jt][:, 512:SR[r]],
                                    start=False, stop=last)

                        if pend is not None:
                            pend()
                        pend = (lambda it=it, ps=ps: post_it(r, it, ps))
                    pend()

                for g in range(GPC):
                    emit_pool(g)

            for _rep in range(body_reps):
                emit_body()

    nc.compile()
    return nc


def _host_prep(x, edge_index):
    """Build A_T, degree scalings and all per-core constant arrays."""
    ei = np.asarray(edge_index).astype(np.int64)
    x = np.asarray(x).astype(np.float32)

    A = np.zeros(N * N, dtype=np.uint8)
    np.add.at(A, ei[0] * N + ei[1], 1)
    A = A.reshape(N, N)
    A[np.arange(N), np.arange(N)] += 1
    # d = colsum ** -0.5 computed via float64 (bitwise-matches jax cpu pow)
    d1 = (A.sum(axis=0, dtype=np.int64).astype(np.float64) ** -0.5)
    d2 = (A.sum(axis=1, dtype=np.int64).astype(np.float64) ** -0.5)
    AT = (A + A.T).astype(BF16)                # entries are small ints: exact
    del A
    return x, AT, d1.astype(np.float32), d2.astype(np.float32)


def _concat_inputs(x, AT, d1, d2, W1, b1, W2, b2, W3, b3):
    """Concatenated (core-major on dim0) input arrays, minimal copies."""
    def f32(a):
        return np.asarray(a).astype(np.float32)

    w1 = f32(W1)
    w2 = f32(W2)
    w1t = np.ascontiguousarray(w1.transpose(0, 2, 1))
    w2t = np.ascontiguousarray(w2.transpose(0, 2, 1))
    w3t = np.ascontiguousarray(f32(W3).transpose(0, 2, 1))
    b1bc = np.ascontiguousarray(
        np.broadcast_to(f32(b1)[:, None, :], (L, P, P)))
    b2bc = np.ascontiguousarray(
        np.broadcast_to(f32(b2)[:, None, :], (L, P, P)))
    b3bc = np.ascontiguousarray(
        np.broadcast_to(f32(b3)[:, None, :], (L, P, P)))
    ident = np.eye(P, dtype=np.float32)
    iotar = np.ascontiguousarray(
        np.broadcast_to(np.arange(P, dtype=np.float32)[None, :], (P, P)))
    fidx = np.arange(NPG)[None, :]
    ltc = np.ascontiguousarray(np.stack([
        (fidx < (t * P + np.arange(P)[:, None])).astype(np.float32)
        for t in range(2)]))
    ones1 = np.ones((1, P), dtype=np.float32)

    # host-computed stage 0: [S1 | S2h | S2l] with S2 split bf16 hi/lo
    s1 = (d1[:, None] * (x @ w1[0].T)).astype(np.float32)
    s2 = (d2[:, None] * (x @ w2[0].T)).astype(np.float32)
    s0 = np.empty((N, 3 * P), dtype=BF16)
    s0[:, 0:P] = s1.astype(BF16)
    s0[:, P:2 * P] = s2.astype(BF16)
    s0[:, 2 * P:3 * P] = (s2 - s0[:, P:2 * P].astype(np.float32)).astype(BF16)

    # [jt, j_in, c, it, i_in] -> [c, it, j_in, jt, i_in]
    at_all = np.ascontiguousarray(
        AT.reshape(JT, P, NC, IT, P).transpose(2, 3, 1, 0, 4))
    d1m = np.ascontiguousarray(
        d1.reshape(NC, IT, P).transpose(0, 2, 1)).reshape(NC * P, IT)
    d2m = np.ascontiguousarray(
        d2.reshape(NC, IT, P).transpose(0, 2, 1)).reshape(NC * P, IT)

    def tile8(a):
        return np.ascontiguousarray(
            np.broadcast_to(a[None], (NC, *a.shape))).reshape(
                NC * a.shape[0], *a.shape[1:])

    return {
        "at": at_all.reshape(NC * IT, P, JT * P),
        "s0": tile8(s0),
        "w1t": tile8(w1t), "w2t": tile8(w2t), "w3t": tile8(w3t),
        "b1bc": tile8(b1bc), "b2bc": tile8(b2bc), "b3bc": tile8(b3bc),
        "d1m": d1m, "d2m": d2m,
        "ident": tile8(ident), "iotar": tile8(iotar),
        "ltc": tile8(ltc), "ones1": tile8(ones1),
    }


def _get_runner(**build_kw):
    """Build (once) a cached jitted SPMD callable for the compiled program."""
    key = ("runner",) + tuple(sorted(build_kw.items()))
    if key in _CACHE:
        return _CACHE[key]
    import jax
    import concourse.mybir as mybir
    from concourse.bass2jax import (
        _bass_exec_p, install_neuronx_cc_hook, partition_id_tensor)
    from jax.experimental.shard_map import shard_map
    from jax.sharding import Mesh, PartitionSpec, NamedSharding

    install_neuronx_cc_hook()
    nckey = ("nc",) + tuple(sorted(build_kw.items()))
    if nckey not in _CACHE:
        _CACHE[nckey] = _build_nc(**build_kw)
    nc = _CACHE[nckey]

    part_name = (nc.partition_id_tensor.name
                 if nc.partition_id_tensor else None)
    in_names, out_names, out_avals, zero_outs = [], [], [], []
    for alloc in nc.m.functions[0].allocations:
        if not isinstance(alloc, mybir.MemoryLocationSet):
            continue
        name = alloc.memorylocations[0].name
        if alloc.kind == "ExternalInput":
            if name != part_name:
                in_names.append(name)
        elif alloc.kind == "ExternalOutput":
            out_names.append(name)
            shape = tuple(alloc.tensor_shape)
            dtype = mybir.dt.np(alloc.dtype)
            out_avals.append(jax.core.ShapedArray(shape, dtype))
            zero_outs.append((shape, dtype))
    n_params = len(in_names)
    n_outs = len(out_names)
    all_names = in_names + out_names
    if part_name is not None:
        all_names = all_names + [part_name]

    def _body(*args):
        operands = list(args)
        if part_name is not None:
            operands.append(partition_id_tensor())
        outs = _bass_exec_p.bind(
            *operands,
            out_avals=tuple(out_avals),
            in_names=tuple(all_names),
            out_names=tuple(out_names),
            lowering_input_output_aliases=(),
            sim_require_finite=True,
            sim_require_nnan=True,
            nc=nc,
        )
        return tuple(outs)

    devices = jax.devices()[:NC]
    mesh = Mesh(np.asarray(devices), ("core",))
    spec = PartitionSpec("core")
    donate = tuple(range(n_params, n_params + n_outs))
    fn = jax.jit(
        shard_map(_body, mesh=mesh, in_specs=(spec,) * (n_params + n_outs),
                  out_specs=(spec,) * n_outs, check_rep=False),
        donate_argnums=donate, keep_unused=True)
    sharding = NamedSharding(mesh, spec)
    runner = {
        "fn": fn, "in_names": in_names, "out_names": out_names,
        "zero_outs": zero_outs, "sharding": sharding, "jax": jax,
    }
    _CACHE[key] = runner
    return runner


def _prep_device_inputs(inputs, **build_kw):
    """Host prep + upload per-core inputs to the devices."""
    r = _get_runner(**build_kw)
    jax = r["jax"]
    x, AT, d1, d2 = _host_prep(inputs["x"], inputs["edge_index"])
    concat_map = _concat_inputs(x, AT, d1, d2,
                                inputs["W1"], inputs["b1"],
                                inputs["W2"], inputs["b2"],
                                inputs["W3"], inputs["b3"])
    dev_in = [jax.device_put(concat_map[nm], r["sharding"])
              for nm in r["in_names"]]
    for a in dev_in:
        a.block_until_ready()
    return dev_in


def _exec(dev_in, **build_kw):
    r = _get_runner(**build_kw)
    jax = r["jax"]
    zeros = [jax.device_put(np.zeros((NC * s[0], *s[1:]), d), r["sharding"])
             for s, d in r["zero_outs"]]
    for z in zeros:
        z.block_until_ready()
    import time
    t0 = time.perf_counter()
    outs = r["fn"](*dev_in, *zeros)
    outs = jax.block_until_ready(outs)
    t1 = time.perf_counter()
    return outs, (t1 - t0)


def _run(inputs, reps=1, **build_kw):
    r = _get_runner(**build_kw)
    dev_in = _prep_device_inputs(inputs, **build_kw)
    times = []
    outs = None
    for _ in range(max(1, reps)):
        outs, dt = _exec(dev_in, **build_kw)
        times.append(dt)
    arr = np.asarray(outs[r["out_names"].index("out")])
    pooled = arr.reshape(NC, GPC * K, D).reshape(G, K, D)
    return np.ascontiguousarray(pooled.astype(np.float32)), times


def kernel(**inputs) -> np.ndarray:
    out, _ = _run(inputs, reps=1)
    return out

